# revision 23
# baseline (speedup 1.0000x reference)
"""GPDconv (GNN message passing) Trainium2 Bass kernel — PE one-hot design.

Batch-parallel over 8 NeuronCores (one batch per core). The previous design
spent ~4ms/core in Q7 SWDGE descriptor generation (~8ns per gather index,
~500k indices). This version keeps exactly TWO per-edge SWDGE passes (the
provable floor) and does all aggregation on the PE via one-hot matmuls:

  sigma1: edges sorted into 32 host-balanced target-blocks (128 ega-targets,
    exactly 4096 edges each). One dma_gather of x pair-rows per edge
    (+ ~6% slot padding from the rnorm partition constraint). Per 128-edge
    group: V1 = u*rnorm*x_row, one-hot over within-block target -> PE matmul
    accumulating x_hat^T [32ch, 128t] in PSUM. rnorm[p] is delivered by a
    96-plane select: edge partition q == (p + rot_c) % 128 for one of three
    rotations (3-choice load balancing), rnorm planes live at [q, 32c+j].
  C: y = (x_hat @ W) . D^T per 128-target tile (targets in permuted order).
  sigma2: edges sorted into 256 host-balanced pair-blocks (128 node-pairs,
    exactly 512 edges each). One dma_gather of y rows per edge (zero pad).
    V2 = gauss*(parity masks)*y, one-hot over within-block pair -> PE matmul
    -> out pair-rows [128, 64] per block, written permuted; host unpermutes.

Host does index/layout prep only (sorting, balancing, packing, int16);
all value math (gauss, norms, products, reductions) runs on device.
"""
import sys

if '/opt/trn_rl_repo' not in sys.path:
    sys.path.insert(0, '/opt/trn_rl_repo')

import numpy as np
import concourse.bacc as bacc
import concourse.mybir as mybir
import concourse.tile as tile
from concourse import bass_utils, library_config, masks

f32 = mybir.dt.float32
f16 = mybir.dt.float16
i16 = mybir.dt.int16

CFG = dict(N=65536, NUM_PTS=4096, K=32, CIN=32, COUT=32, KM=16,
           G1FIX=36, ROTS=(0, 43), S2CHUNK=8)

mult, add, subtract = (mybir.AluOpType.mult, mybir.AluOpType.add,
                       mybir.AluOpType.subtract)
is_equal = mybir.AluOpType.is_equal
Exp = mybir.ActivationFunctionType.Exp
X = mybir.AxisListType.X


def _wrap16(a):
    return np.ascontiguousarray(np.tile(a.reshape(-1, 16).T, (8, 1)))


def _balance_blocks(deg, nblocks, per_block_items, per_block_sum):
    """Partition items into nblocks of exactly per_block_items items with
    degree sums exactly per_block_sum. Snake-deal + exact swap repair."""
    deg = np.asarray(deg, np.int64)
    n = len(deg)
    assert n == nblocks * per_block_items
    assert deg.sum() == nblocks * per_block_sum
    order = np.argsort(-deg, kind='stable')
    # snake deal: rows of nblocks, alternating direction
    rows = order.reshape(per_block_items, nblocks)
    for r in range(1, per_block_items, 2):
        rows[r] = rows[r][::-1]
    blocks = [list(rows[:, b]) for b in range(nblocks)]
    sums = np.array([deg[b].sum() for b in blocks], np.int64)
    for _ in range(100000):
        dev = sums - per_block_sum
        if not dev.any():
            break
        hi = int(np.argmax(dev))
        lo = int(np.argmin(dev))
        dstar = int(min(dev[hi], -dev[lo]))
        ha = np.asarray(blocks[hi])
        la = np.asarray(blocks[lo])
        da, db = deg[ha], deg[la]
        ua = np.unique(da)
        ub = np.unique(db)
        found = None
        for want in range(dstar, 0, -1):
            hit = ua[np.isin(ua - want, ub)]
            if len(hit):
                va = int(hit[0])
                ai = int(np.nonzero(da == va)[0][0])
                bj = int(np.nonzero(db == va - want)[0][0])
                found = (ai, bj, want)
                break
        assert found is not None, (dev[hi], dev[lo], ua, ub)
        ai, bj, want = found
        a_it, b_it = int(ha[ai]), int(la[bj])
        blocks[hi][ai] = b_it
        blocks[lo][bj] = a_it
        sums[hi] -= want
        sums[lo] += want
    assert (sums == per_block_sum).all(), sums
    return [np.asarray(b, np.int64) for b in blocks]


def _assign_bins(res, rots, cap):
    """3-choice capacitated assignment: edge i may go to bin
    (res[i]+rot)%128; return bin per edge with loads <= cap.
    Greedy lightest-bin init + BFS augmenting-path eviction."""
    n = len(res)
    nr = len(rots)
    cands = np.stack([(res + r) % 128 for r in rots], 1)   # (n, nr)
    cnt = np.zeros(128, np.int64)
    choice = np.zeros(n, np.int64)
    order = np.random.default_rng(0).permutation(n)
    for i in order:
        c = cands[i]
        j = int(np.argmin(cnt[c]))
        choice[i] = j
        cnt[c[j]] += 1
    # bin -> member edge list
    members = [[] for _ in range(128)]
    for i in range(n):
        members[int(cands[i, choice[i]])].append(i)
    while True:
        over = [b for b in range(128) if cnt[b] > cap]
        if not over:
            break
        s = over[0]
        # BFS from s to any bin with load < cap via edge reassignments
        parent = {s: None}
        frontier = [s]
        goal = None
        while frontier and goal is None:
            nxt = []
            for u in frontier:
                for i in members[u]:
                    for j in range(nr):
                        v = int(cands[i, j])
                        if v == u or v in parent:
                            continue
                        parent[v] = (u, i, j)
                        if cnt[v] < cap:
                            goal = v
                            break
                        nxt.append(v)
                    if goal is not None:
                        break
                if goal is not None:
                    break
            frontier = nxt
        assert goal is not None, "no augmenting path; raise G1FIX"
        # walk back, reassigning one edge per hop
        v = goal
        while parent[v] is not None:
            u, i, j = parent[v]
            members[u].remove(i)
            members[v].append(i)
            choice[i] = j
            cnt[u] -= 1
            cnt[v] += 1
            v = u
    assert cnt.max() <= cap, (cnt.max(), cap)
    return cands[np.arange(n), choice]


def host_prep(cfg, x_b, grid_b, gw_b, eg_b, ega_b, basepts, base_weight, D,
              weights):
    N, NUM_PTS, K = cfg["N"], cfg["NUM_PTS"], cfg["K"]
    CIN, COUT, KM = cfg["CIN"], cfg["COUT"], cfg["KM"]
    G1FIX, ROTS = cfg["G1FIX"], cfg["ROTS"]
    E = K * NUM_PTS
    PCOLS = NUM_PTS // 128
    eg = eg_b.T.reshape(-1).astype(np.int64)        # (E,) [k, p] order
    ega = ega_b.T.reshape(-1).astype(np.int64)
    pp = np.tile(np.arange(NUM_PTS), K)

    # ---------------- xcat pair-row table ----------------
    rows = np.zeros((N, 64), np.float32)
    rows[:, :CIN] = x_b.T
    rows[:, CIN] = grid_b[:, 0]
    rows[:, CIN + 1] = grid_b[:, 1]
    rows[:, CIN + 2] = gw_b
    xcat = rows.astype(np.float16).reshape(N // 2, 128)

    # ---------------- dense tab (rnorm pass) ----------------
    def lay_dense(v):
        return np.ascontiguousarray(
            v.reshape(K, PCOLS, 128).transpose(2, 1, 0).reshape(128, E // 128))
    dtab = np.stack([
        lay_dense(grid_b[eg, 0].reshape(K, NUM_PTS)),
        lay_dense(grid_b[eg, 1].reshape(K, NUM_PTS)),
        lay_dense(gw_b[eg].reshape(K, NUM_PTS)),
        lay_dense(basepts[ega, 0].reshape(K, NUM_PTS)),
        lay_dense(basepts[ega, 1].reshape(K, NUM_PTS)),
    ], axis=-1).astype(np.float16)
    bwd = np.stack([base_weight[:, 0].reshape(PCOLS, 128).T,
                    base_weight[:, 1].reshape(PCOLS, 128).T], axis=-1)

    # ---------------- sigma1: balanced target blocks ----------------
    tdeg = np.bincount(ega, minlength=NUM_PTS)
    blocks1 = _balance_blocks(tdeg, 32, 128, E // 32)
    t_newrow = np.empty(NUM_PTS, np.int64)          # orig target -> new row
    t_local = np.empty(NUM_PTS, np.int64)
    t_block = np.empty(NUM_PTS, np.int64)
    for b in range(32):
        t_newrow[blocks1[b]] = 128 * b + np.arange(128)
        t_local[blocks1[b]] = np.arange(128)
        t_block[blocks1[b]] = b

    SG1 = 32 * G1FIX
    S1 = SG1 * 128
    xidx1 = np.zeros(S1, np.int16)
    tab1 = np.zeros((S1, 8), np.float16)            # bpx bpy bwx bwy me mo egar prow
    tab1[:, 6] = -1.0
    tab1[:, 7] = 127.0                              # no plane match for holes
    for b in range(32):
        sel = np.nonzero(t_block[ega] == b)[0]
        assert len(sel) == E // 32
        res = pp[sel] % 128
        q = _assign_bins(res, ROTS, G1FIX)
        # slot within block: (q, g) with g = rank within bin q
        order = np.argsort(q, kind='stable')
        sel, q = sel[order], q[order]
        cnt = np.bincount(q, minlength=128)
        starts = np.concatenate([[0], np.cumsum(cnt)])[:-1]
        g = np.arange(len(sel)) - starts[q]
        slot = (b * G1FIX + g) * 128 + q
        xidx1[slot] = (eg[sel] >> 1).astype(np.int16)
        tab1[slot, 0] = basepts[ega[sel], 0]
        tab1[slot, 1] = basepts[ega[sel], 1]
        tab1[slot, 2] = base_weight[pp[sel], 0]
        tab1[slot, 3] = base_weight[pp[sel], 1]
        tab1[slot, 4] = (1 - (eg[sel] & 1)).astype(np.float16)
        tab1[slot, 5] = (eg[sel] & 1).astype(np.float16)
        tab1[slot, 6] = t_local[ega[sel]].astype(np.float16)
        rot_used = (q - pp[sel]) % 128
        cidx = np.zeros(len(sel), np.int64)
        for ci, r in enumerate(ROTS):
            cidx[rot_used == r] = ci
        tab1[slot, 7] = (cidx * 32 + (pp[sel] >> 7)).astype(np.float16)

    # tab1 device layout: [128, 8, SG1] (plane-major per partition)
    tab1_dev = np.ascontiguousarray(
        tab1.reshape(SG1, 128, 8).transpose(1, 2, 0)).astype(np.float16)

    # rotation matrices for rnorm planes (f16): R[q, q'] = [q' == (q+rot)%128]
    NROT = len(ROTS) - 1
    rotm = np.zeros((NROT, 128, 128), np.float16)
    for ci, r in enumerate(ROTS[1:]):
        rotm[ci, np.arange(128), (np.arange(128) + r) % 128] = 1.0

    # ---------------- sigma2: balanced pair blocks ----------------
    m2 = eg >> 1
    pdeg = np.bincount(m2, minlength=N // 2)
    blocks2 = _balance_blocks(pdeg, 256, 128, E // 256)
    p_local = np.empty(N // 2, np.int64)
    p_block = np.empty(N // 2, np.int64)
    p_newrow = np.empty(N // 2, np.int64)
    for b in range(256):
        p_local[blocks2[b]] = np.arange(128)
        p_block[blocks2[b]] = b
        p_newrow[blocks2[b]] = 128 * b + np.arange(128)

    SG2 = 1024
    S2 = SG2 * 128
    yidx2 = np.zeros(S2, np.int16)
    tab2 = np.zeros((S2, 8), np.float16)            # gx gy bpx bpy bwx bwy gme gmo... see below
    tab2[:, 7] = -1.0                               # prel hole marker unused (masks=0)
    slot2_of = np.empty(E, np.int64)
    pos = 0
    for b in range(256):
        sel = np.nonzero(p_block[m2] == b)[0]
        assert len(sel) == E // 256
        n = len(sel)
        slot = pos + np.arange(n)
        pos += n
        yidx2[slot] = t_newrow[ega[sel]].astype(np.int16)
        tab2[slot, 0] = grid_b[eg[sel], 0]
        tab2[slot, 1] = grid_b[eg[sel], 1]
        tab2[slot, 2] = basepts[ega[sel], 0]
        tab2[slot, 3] = basepts[ega[sel], 1]
        tab2[slot, 4] = base_weight[pp[sel], 0]
        tab2[slot, 5] = base_weight[pp[sel], 1]
        # plane 6 = prel (pair within block), plane 7 = even-node mask
        tab2[slot, 6] = p_local[m2[sel]].astype(np.float16)
        tab2[slot, 7] = (1 - (eg[sel] & 1)).astype(np.float16)
        slot2_of[sel] = slot
    tab2_dev = np.ascontiguousarray(
        tab2.reshape(SG2, 128, 8).transpose(1, 2, 0)).astype(np.float16)

    # host finish: orig pair row = out_tbl[p_newrow[pair]]

    # dtt rows permuted by target new-row
    t_origin = np.empty(NUM_PTS, np.int64)
    t_origin[t_newrow] = np.arange(NUM_PTS)
    dtt = np.ascontiguousarray(D.T[t_origin].astype(np.float32))

    # tiled iota: [128, G1FIX*128], content[q, g*128+j] = j (one materialized
    # copy per group column so one-hot is_eq needs no stride-0 inner operand)
    iota_row = np.tile(np.arange(128, dtype=np.float16)[None, None, :],
                       (128, G1FIX, 1)).reshape(128, G1FIX * 128)

    return dict(
        xcat=xcat,
        dtab=dtab,
        bwd=np.ascontiguousarray(bwd.astype(np.float32)),
        wfl=np.ascontiguousarray(weights.reshape(CIN, COUT * KM).astype(np.float32)),
        dt_t=dtt,
        xidx1=_wrap16(xidx1),
        tab1=tab1_dev.reshape(128, 8 * SG1),
        rotm=np.ascontiguousarray(rotm.reshape(NROT * 128, 128)),
        yidx2=_wrap16(yidx2),
        tab2=tab2_dev.reshape(128, 8 * SG2),
        iota=np.ascontiguousarray(iota_row),
    ), p_newrow


def build(nc, cfg):
    N, NUM_PTS, K = cfg["N"], cfg["NUM_PTS"], cfg["K"]
    CIN, COUT, KM = cfg["CIN"], cfg["COUT"], cfg["KM"]
    G1FIX = cfg["G1FIX"]
    S2CHUNK = cfg["S2CHUNK"]
    E = K * NUM_PTS
    PCOLS = NUM_PTS // 128
    OJ = COUT * KM
    SG1 = 32 * G1FIX
    SG2 = 1024
    NPLANES = 32 * len(cfg["ROTS"])

    xcat_d = nc.dram_tensor("xcat", [N // 2, 128], f16, kind="ExternalInput")
    dtab_d = nc.dram_tensor("dtab", [128, E // 128, 5], f16, kind="ExternalInput")
    bwd_d = nc.dram_tensor("bwd", [128, PCOLS, 2], f32, kind="ExternalInput")
    wfl_d = nc.dram_tensor("wfl", [CIN, OJ], f32, kind="ExternalInput")
    dtt_d = nc.dram_tensor("dt_t", [NUM_PTS, KM], f32, kind="ExternalInput")
    xidx1_d = nc.dram_tensor("xidx1", [128, SG1 * 128 // 16], i16, kind="ExternalInput")
    tab1_d = nc.dram_tensor("tab1", [128, 8 * SG1], f16, kind="ExternalInput")
    NROT = len(cfg["ROTS"]) - 1
    rotm_d = nc.dram_tensor("rotm", [NROT * 128, 128], f16, kind="ExternalInput")
    yidx2_d = nc.dram_tensor("yidx2", [128, SG2 * 128 // 16], i16, kind="ExternalInput")
    tab2_d = nc.dram_tensor("tab2", [128, 8 * SG2], f16, kind="ExternalInput")
    iota_d = nc.dram_tensor("iota", [128, 128 * cfg["G1FIX"]], f16, kind="ExternalInput")
    out_d = nc.dram_tensor("out", [N // 2, 64], f32, kind="ExternalOutput")
    ycat_d = nc.dram_tensor("ycat_tbl", [NUM_PTS, 64], f32, kind="Internal")

    with tile.TileContext(nc) as tc:
        with tc.tile_pool(name="consts", bufs=1) as cp:
            ident = cp.tile([128, 128], f32)
            masks.make_identity(nc, ident[:])
            nc.gpsimd.load_library(library_config.mlp)

            wfl = cp.tile([CIN, OJ], f32)
            nc.sync.dma_start(wfl[:], wfl_d[:])
            bwd = cp.tile([128, PCOLS * 2], f32)
            bwd3 = bwd[:].rearrange("p (q t) -> p q t", t=2)
            nc.sync.dma_start(bwd3, bwd_d[:])
            iota = cp.tile([128, 128 * G1FIX], f16)
            nc.sync.dma_start(iota[:], iota_d[:])
            rotm = cp.tile([128, NROT * 128], f16)
            nc.sync.dma_start(rotm[:].rearrange("p (c j) -> p c j", c=NROT),
                              rotm_d.ap().rearrange("(c p) j -> p c j", p=128))
            tab1 = cp.tile([128, 8 * SG1], f16)
            nc.sync.dma_start(tab1[:], tab1_d[:])
            tab13 = tab1[:].rearrange("p (t s) -> p t s", t=8)
            xi1 = cp.tile([128, SG1 * 8], i16)
            nc.sync.dma_start(xi1[:], xidx1_d[:])
            yi2 = cp.tile([128, SG2 * 8], i16)
            nc.sync.dma_start(yi2[:], yidx2_d[:])


            # ---------- dense pass: rnorm planes (cp pool: no SBUF reuse
            # so sigma1 gathers are not blocked behind this) ----------
            rnt = cp.tile([128, NPLANES], f16)
            with tc.tile_pool(name="dpsum", bufs=1, space="PSUM") as dq:
                JD = E // 128
                dtab = cp.tile([128, JD * 5], f16)
                dt3 = dtab[:].rearrange("p (j t) -> p j t", t=5)
                nc.sync.dma_start(dt3, dtab_d[:])
                dd0 = cp.tile([128, JD], f32)
                dd1 = cp.tile([128, JD], f32)
                nc.vector.tensor_tensor(dd0[:], dt3[:, :, 0], dt3[:, :, 3], op=subtract)
                nc.vector.tensor_tensor(dd0[:], dd0[:], dd0[:], op=mult)
                nc.vector.tensor_tensor(dd1[:], dt3[:, :, 1], dt3[:, :, 4], op=subtract)
                nc.vector.tensor_tensor(dd1[:], dd1[:], dd1[:], op=mult)
                d0k = dd0[:].rearrange("p (q k) -> p q k", k=K)
                d1k = dd1[:].rearrange("p (q k) -> p q k", k=K)
                nc.vector.tensor_tensor(d0k, d0k,
                                        bwd3[:, :, 0].broadcast_to((128, PCOLS, K)),
                                        op=mult)
                nc.vector.tensor_tensor(d1k, d1k,
                                        bwd3[:, :, 1].broadcast_to((128, PCOLS, K)),
                                        op=mult)
                nc.vector.tensor_tensor(dd0[:], dd0[:], dd1[:], op=add)
                nc.scalar.activation(dd1[:], dd0[:], Exp, scale=-1.0)
                nc.vector.tensor_tensor(dd1[:], dd1[:], dt3[:, :, 2], op=mult)
                nc.vector.tensor_tensor(dd1[:], dd1[:], dd1[:], op=mult)
                nsq = cp.tile([128, PCOLS], f32)
                nc.vector.reduce_sum(nsq[:].unsqueeze(2),
                                     dd1[:].rearrange("p (q k) -> p q k", k=K),
                                     axis=X)
                nc.scalar.activation(nsq[:], nsq[:],
                                     mybir.ActivationFunctionType.Sqrt)
                nc.vector.tensor_scalar_add(nsq[:], nsq[:], 1e-5)
                nc.vector.reciprocal(nsq[:], nsq[:])
                nc.vector.tensor_copy(rnt[:, 0:32], nsq[:])
                for ci in range(NROT):
                    rp = dq.tile([128, 32], f32, tag="rp")
                    nc.tensor.matmul(rp[:], rotm[:, ci * 128:(ci + 1) * 128],
                                     rnt[:, 0:32], start=True, stop=True)
                    nc.vector.tensor_copy(rnt[:, 32 + 32 * ci:64 + 32 * ci], rp[:])

            # ---------- rn_all: 96-plane select (4 chunks so sigma1's early
            # blocks unblock before the whole select finishes) ----------
            rn_all = cp.tile([128, SG1], f16)
            rtmp = cp.tile([128, SG1], f16)
            nc.vector.memset(rn_all[:], 0.0)
            prow = tab13[:, 7, :]
            RC = SG1 // 4
            for r in range(4):
                cs = slice(r * RC, (r + 1) * RC)
                for j in range(NPLANES):
                    nc.vector.scalar_tensor_tensor(
                        rtmp[:, cs], prow[:, cs], float(j),
                        rnt[:, j:j + 1].broadcast_to((128, RC)),
                        op0=is_equal, op1=mult)
                    nc.vector.tensor_tensor(rn_all[:, cs], rn_all[:, cs],
                                            rtmp[:, cs], op=add)

            # ---------- sigma1 + fused phase C ----------
            xhT = cp.tile([CIN, NUM_PTS], f32)
            with tc.tile_pool(name="ph1", bufs=3) as p1, \
                    tc.tile_pool(name="ps1", bufs=2, space="PSUM") as q1:
                for b in range(32):
                    sl = slice(b * G1FIX, (b + 1) * G1FIX)
                    gx = p1.tile([128, G1FIX * 128], f16, tag="gx", bufs=4)
                    gx3 = gx[:].rearrange("p (g e) -> p g e", e=128)
                    nc.gpsimd.dma_gather(
                        gx3, xcat_d[:],
                        xi1[:, b * G1FIX * 8:(b + 1) * G1FIX * 8],
                        G1FIX * 128, G1FIX * 128, 128,
                        elem_step=128, single_packet=False,
                        queue_num=b % 4)
                    me = tab13[:, 4, sl]
                    mo = tab13[:, 5, sl]
                    # grid/gw of the edge's node via parity select
                    ge = p1.tile([128, G1FIX * 3], f32, tag="ge")
                    ge3 = ge[:].rearrange("p (g t) -> p g t", t=3)
                    t0 = p1.tile([128, G1FIX * 3], f32, tag="t0")
                    t03 = t0[:].rearrange("p (g t) -> p g t", t=3)
                    nc.vector.tensor_tensor(
                        ge3, gx3[:, :, 32:35],
                        me.unsqueeze(2).broadcast_to((128, G1FIX, 3)), op=mult)
                    nc.vector.tensor_tensor(
                        t03, gx3[:, :, 96:99],
                        mo.unsqueeze(2).broadcast_to((128, G1FIX, 3)), op=mult)
                    nc.vector.tensor_tensor(ge3, ge3, t03, op=add)
                    dd = p1.tile([128, G1FIX * 2], f32, tag="dd")
                    dd3 = dd[:].rearrange("p (g t) -> p g t", t=2)
                    nc.vector.tensor_tensor(
                        dd3, ge3[:, :, 0:2],
                        tab13[:, 0:2, sl].rearrange("p t s -> p s t"), op=subtract)
                    nc.vector.tensor_tensor(dd3, dd3, dd3, op=mult)
                    nc.vector.tensor_tensor(
                        dd3, dd3,
                        tab13[:, 2:4, sl].rearrange("p t s -> p s t"), op=mult)
                    ga = p1.tile([128, G1FIX], f32, tag="ga")
                    nc.vector.tensor_tensor(ga[:], dd3[:, :, 0], dd3[:, :, 1],
                                            op=add)
                    nc.scalar.activation(ga[:], ga[:], Exp, scale=-1.0)
                    nc.vector.tensor_tensor(ga[:], ga[:], ge3[:, :, 2], op=mult)
                    nc.vector.tensor_tensor(ga[:], ga[:], rn_all[:, sl], op=mult)
                    wlo = p1.tile([128, G1FIX], f32, tag="wlo")
                    whi = p1.tile([128, G1FIX], f32, tag="whi")
                    nc.vector.tensor_tensor(wlo[:], ga[:], me, op=mult)
                    nc.vector.tensor_tensor(whi[:], ga[:], mo, op=mult)
                    v1 = p1.tile([128, G1FIX * CIN], f16, tag="v1")
                    v13 = v1[:].rearrange("p (g e) -> p g e", e=CIN)
                    t1 = p1.tile([128, G1FIX * CIN], f16, tag="t1")
                    t13 = t1[:].rearrange("p (g e) -> p g e", e=CIN)
                    nc.vector.tensor_tensor(
                        v13, gx3[:, :, 0:CIN],
                        wlo[:].unsqueeze(2).broadcast_to((128, G1FIX, CIN)),
                        op=mult)
                    nc.vector.tensor_tensor(
                        t13, gx3[:, :, 64:64 + CIN],
                        whi[:].unsqueeze(2).broadcast_to((128, G1FIX, CIN)),
                        op=mult)
                    nc.vector.tensor_tensor(v13, v13, t13, op=add)
                    oh = p1.tile([128, G1FIX * 128], f16, tag="oh")
                    oh3 = oh[:].rearrange("p (g e) -> p g e", e=128)
                    nc.vector.tensor_tensor(
                        oh3,
                        tab13[:, 6, sl].unsqueeze(2).broadcast_to((128, G1FIX, 128)),
                        iota[:].rearrange("p (g e) -> p g e", e=128),
                        op=is_equal)
                    ps = q1.tile([CIN, 128], f32, tag="pxh")
                    for g in range(G1FIX):
                        nc.tensor.matmul(ps[:], v13[:, g, :], oh3[:, g, :],
                                         start=(g == 0), stop=(g == G1FIX - 1))
                    nc.vector.tensor_copy(xhT[:, b * 128:(b + 1) * 128], ps[:])
                    # fused phase C for this 128-target tile
                    o1p = q1.tile([128, OJ], f32, tag="o1p")
                    nc.tensor.matmul(o1p[:], xhT[:, b * 128:(b + 1) * 128],
                                     wfl[:], start=True, stop=True)
                    dtt = p1.tile([128, KM], f32, tag="dtt")
                    nc.sync.dma_start(dtt[:], dtt_d[b * 128:(b + 1) * 128, :])
                    o1 = p1.tile([128, OJ], f32, tag="o1")
                    nc.vector.tensor_tensor(
                        o1[:].rearrange("p (o j) -> p o j", j=KM),
                        o1p[:].rearrange("p (o j) -> p o j", j=KM),
                        dtt[:].unsqueeze(1).broadcast_to((128, COUT, KM)),
                        op=mult)
                    yrow = p1.tile([128, 64], f32, tag="yrow")
                    nc.vector.reduce_sum(
                        yrow[:, 0:COUT].unsqueeze(2),
                        o1[:].rearrange("p (o j) -> p o j", j=KM), axis=X)
                    nc.sync.dma_start(
                        ycat_d.ap()[b * 128:(b + 1) * 128, 0:COUT], yrow[:, 0:COUT])

            # ---------- sigma2 ----------
            NCH = 256 // S2CHUNK          # chunks
            GC = S2CHUNK * 4              # group-columns per chunk (G2FIX=4)
            with tc.tile_pool(name="ph2", bufs=2) as p2, \
                    tc.tile_pool(name="ps2", bufs=2, space="PSUM") as q2:
                tab2 = p2.tile([128, 8 * SG2], f16, tag="tab2", bufs=1)
                nc.sync.dma_start(tab2[:], tab2_d[:])
                tab23 = tab2[:].rearrange("p (t s) -> p t s", t=8)
                for c in range(NCH):
                    s0 = c * GC           # first group-col of chunk
                    sl = slice(s0, s0 + GC)
                    gy = p2.tile([128, GC * 64], f32, tag="gy", bufs=4)
                    gy3 = gy[:].rearrange("p (g e) -> p g e", e=64)
                    nc.gpsimd.dma_gather(
                        gy3, ycat_d[:],
                        yi2[:, s0 * 8:(s0 + GC) * 8],
                        GC * 128, GC * 128, 64,
                        elem_step=64, single_packet=False,
                        queue_num=c % 4)
                    dd = p2.tile([128, GC * 2], f32, tag="dd2")
                    dd3 = dd[:].rearrange("p (g t) -> p g t", t=2)
                    nc.vector.tensor_tensor(
                        dd3, tab23[:, 0:2, sl].rearrange("p t s -> p s t"),
                        tab23[:, 2:4, sl].rearrange("p t s -> p s t"), op=subtract)
                    nc.vector.tensor_tensor(dd3, dd3, dd3, op=mult)
                    nc.vector.tensor_tensor(
                        dd3, dd3,
                        tab23[:, 4:6, sl].rearrange("p t s -> p s t"), op=mult)
                    ga = p2.tile([128, GC], f32, tag="ga2")
                    nc.vector.tensor_tensor(ga[:], dd3[:, :, 0], dd3[:, :, 1],
                                            op=add)
                    nc.scalar.activation(ga[:], ga[:], Exp, scale=-1.0)
                    me2 = tab23[:, 7, sl]
                    gme = p2.tile([128, GC], f32, tag="gme")
                    gmo = p2.tile([128, GC], f32, tag="gmo")
                    nc.vector.tensor_tensor(gme[:], ga[:], me2, op=mult)
                    nc.vector.tensor_tensor(gmo[:], ga[:], gme[:], op=subtract)
                    v2e = p2.tile([128, GC * 32], f16, tag="v2e")
                    v2e3 = v2e[:].rearrange("p (g e) -> p g e", e=32)
                    v2o = p2.tile([128, GC * 32], f16, tag="v2o")
                    v2o3 = v2o[:].rearrange("p (g e) -> p g e", e=32)
                    nc.vector.tensor_tensor(
                        v2e3, gy3[:, :, 0:32],
                        gme[:].unsqueeze(2).broadcast_to((128, GC, 32)), op=mult)
                    nc.vector.tensor_tensor(
                        v2o3, gy3[:, :, 0:32],
                        gmo[:].unsqueeze(2).broadcast_to((128, GC, 32)), op=mult)
                    oh2 = p2.tile([128, GC * 128], f16, tag="oh2")
                    oh23 = oh2[:].rearrange("p (g e) -> p g e", e=128)
                    nc.vector.tensor_tensor(
                        oh23,
                        tab23[:, 6, sl].unsqueeze(2).broadcast_to((128, GC, 128)),
                        iota[:, :GC * 128].rearrange("p (g e) -> p g e", e=128),
                        op=is_equal)
                    ob = p2.tile([128, S2CHUNK * 64], f32, tag="ob")
                    ob3 = ob[:].rearrange("p (k e) -> p k e", e=64)
                    for k in range(S2CHUNK):
                        po = q2.tile([128, 64], f32, tag="po")
                        po3 = po[:].rearrange("p (h e) -> p h e", e=32)
                        for g in range(4):
                            gc = 4 * k + g
                            nc.tensor.matmul(po3[:, 0, :], oh23[:, gc, :],
                                             v2e3[:, gc, :],
                                             start=(g == 0), stop=(g == 3))
                        for g in range(4):
                            gc = 4 * k + g
                            nc.tensor.matmul(po3[:, 1, :], oh23[:, gc, :],
                                             v2o3[:, gc, :],
                                             start=(g == 0), stop=(g == 3))
                        nc.vector.tensor_copy(ob3[:, k, :], po[:])
                    nc.sync.dma_start(
                        out_d.ap()[c * S2CHUNK * 128:(c + 1) * S2CHUNK * 128, :]
                        .rearrange("(k p) e -> p k e", p=128),
                        ob3)
    return nc


def make_in_maps(cfg, x, grid, grid_weight, edge_grid, edge_Gauss, basepts,
                 base_weight, D, weights):
    maps, invs = [], []
    for b in range(x.shape[0]):
        m, inv2 = host_prep(cfg, x[b], grid[b], grid_weight[b], edge_grid[b],
                            edge_Gauss[b], basepts, base_weight, D, weights)
        maps.append(m)
        invs.append(inv2)
    return maps, invs


def finish(cfg, out_tbl, p_newrow):
    # device row p_newrow[pair] holds pair's output
    o = out_tbl[p_newrow]
    return np.ascontiguousarray(
        o.reshape(cfg["N"], 32)[:, :cfg["COUT"]].T)


_BUILT = {}


def _get_nc():
    if "nc" not in _BUILT:
        nc = bacc.Bacc("TRN2", target_bir_lowering=False,
                       dynamic_dma_scratch_size=32768,
                       num_swdge_queues=4)
        build(nc, CFG)
        nc.compile()
        _BUILT["nc"] = nc
    return _BUILT["nc"]


def kernel(x, grid, grid_weight, edge_grid, edge_Gauss, basepts, base_weight,
           D, weights, _trace=False):
    cfg = CFG
    in_maps, invs = make_in_maps(
        cfg, np.asarray(x, np.float32), np.asarray(grid),
        np.asarray(grid_weight), np.asarray(edge_grid),
        np.asarray(edge_Gauss), np.asarray(basepts),
        np.asarray(base_weight), np.asarray(D), np.asarray(weights))
    nc = _get_nc()
    res = bass_utils.run_bass_kernel_spmd(
        nc, in_maps, core_ids=list(range(x.shape[0])), trace=_trace)
    out = np.stack([finish(cfg, res.results[b]["out"], invs[b])
                    for b in range(x.shape[0])])
    kernel.last_result = res
    return out


# revision 24
# speedup vs baseline: 1.1686x; 1.1686x over previous
"""GPDconv (GNN message passing) Trainium2 Bass kernel — PE one-hot design.

Batch-parallel over 8 NeuronCores (one batch per core). The previous design
spent ~4ms/core in Q7 SWDGE descriptor generation (~8ns per gather index,
~500k indices). This version keeps exactly TWO per-edge SWDGE passes (the
provable floor) and does all aggregation on the PE via one-hot matmuls:

  sigma1: edges sorted into 32 host-balanced target-blocks (128 ega-targets,
    exactly 4096 edges each). One dma_gather of x pair-rows per edge
    (+ ~6% slot padding from the rnorm partition constraint). Per 128-edge
    group: V1 = u*rnorm*x_row, one-hot over within-block target -> PE matmul
    accumulating x_hat^T [32ch, 128t] in PSUM. rnorm[p] is delivered by a
    96-plane select: edge partition q == (p + rot_c) % 128 for one of three
    rotations (3-choice load balancing), rnorm planes live at [q, 32c+j].
  C: y = (x_hat @ W) . D^T per 128-target tile (targets in permuted order).
  sigma2: edges sorted into 256 host-balanced pair-blocks (128 node-pairs,
    exactly 512 edges each). One dma_gather of y rows per edge (zero pad).
    V2 = gauss*(parity masks)*y, one-hot over within-block pair -> PE matmul
    -> out pair-rows [128, 64] per block, written permuted; host unpermutes.

Host does index/layout prep only (sorting, balancing, packing, int16);
all value math (gauss, norms, products, reductions) runs on device.
"""
import sys

if '/opt/trn_rl_repo' not in sys.path:
    sys.path.insert(0, '/opt/trn_rl_repo')

import numpy as np
import concourse.bacc as bacc
import concourse.mybir as mybir
import concourse.tile as tile
from concourse import bass_utils, library_config, masks

f32 = mybir.dt.float32
f16 = mybir.dt.float16
i16 = mybir.dt.int16

CFG = dict(N=65536, NUM_PTS=4096, K=32, CIN=32, COUT=32, KM=16,
           G1FIX=36, ROTS=(0, 43), S2CHUNK=8)

mult, add, subtract = (mybir.AluOpType.mult, mybir.AluOpType.add,
                       mybir.AluOpType.subtract)
is_equal = mybir.AluOpType.is_equal
Exp = mybir.ActivationFunctionType.Exp
X = mybir.AxisListType.X


def _wrap16(a):
    return np.ascontiguousarray(np.tile(a.reshape(-1, 16).T, (8, 1)))


def _balance_blocks(deg, nblocks, per_block_items, per_block_sum):
    """Partition items into nblocks of exactly per_block_items items with
    degree sums exactly per_block_sum. Snake-deal + exact swap repair."""
    deg = np.asarray(deg, np.int64)
    n = len(deg)
    assert n == nblocks * per_block_items
    assert deg.sum() == nblocks * per_block_sum
    order = np.argsort(-deg, kind='stable')
    # snake deal: rows of nblocks, alternating direction
    rows = order.reshape(per_block_items, nblocks)
    for r in range(1, per_block_items, 2):
        rows[r] = rows[r][::-1]
    blocks = [list(rows[:, b]) for b in range(nblocks)]
    sums = np.array([deg[b].sum() for b in blocks], np.int64)
    for _ in range(100000):
        dev = sums - per_block_sum
        if not dev.any():
            break
        hi = int(np.argmax(dev))
        lo = int(np.argmin(dev))
        dstar = int(min(dev[hi], -dev[lo]))
        ha = np.asarray(blocks[hi])
        la = np.asarray(blocks[lo])
        da, db = deg[ha], deg[la]
        ua = np.unique(da)
        ub = np.unique(db)
        found = None
        for want in range(dstar, 0, -1):
            hit = ua[np.isin(ua - want, ub)]
            if len(hit):
                va = int(hit[0])
                ai = int(np.nonzero(da == va)[0][0])
                bj = int(np.nonzero(db == va - want)[0][0])
                found = (ai, bj, want)
                break
        assert found is not None, (dev[hi], dev[lo], ua, ub)
        ai, bj, want = found
        a_it, b_it = int(ha[ai]), int(la[bj])
        blocks[hi][ai] = b_it
        blocks[lo][bj] = a_it
        sums[hi] -= want
        sums[lo] += want
    assert (sums == per_block_sum).all(), sums
    return [np.asarray(b, np.int64) for b in blocks]


def _assign_bins(res, rots, cap):
    """3-choice capacitated assignment: edge i may go to bin
    (res[i]+rot)%128; return bin per edge with loads <= cap.
    Greedy lightest-bin init + BFS augmenting-path eviction."""
    n = len(res)
    nr = len(rots)
    cands = np.stack([(res + r) % 128 for r in rots], 1)   # (n, nr)
    cnt = np.zeros(128, np.int64)
    choice = np.zeros(n, np.int64)
    order = np.random.default_rng(0).permutation(n)
    for i in order:
        c = cands[i]
        j = int(np.argmin(cnt[c]))
        choice[i] = j
        cnt[c[j]] += 1
    # bin -> member edge list
    members = [[] for _ in range(128)]
    for i in range(n):
        members[int(cands[i, choice[i]])].append(i)
    while True:
        over = [b for b in range(128) if cnt[b] > cap]
        if not over:
            break
        s = over[0]
        # BFS from s to any bin with load < cap via edge reassignments
        parent = {s: None}
        frontier = [s]
        goal = None
        while frontier and goal is None:
            nxt = []
            for u in frontier:
                for i in members[u]:
                    for j in range(nr):
                        v = int(cands[i, j])
                        if v == u or v in parent:
                            continue
                        parent[v] = (u, i, j)
                        if cnt[v] < cap:
                            goal = v
                            break
                        nxt.append(v)
                    if goal is not None:
                        break
                if goal is not None:
                    break
            frontier = nxt
        assert goal is not None, "no augmenting path; raise G1FIX"
        # walk back, reassigning one edge per hop
        v = goal
        while parent[v] is not None:
            u, i, j = parent[v]
            members[u].remove(i)
            members[v].append(i)
            choice[i] = j
            cnt[u] -= 1
            cnt[v] += 1
            v = u
    assert cnt.max() <= cap, (cnt.max(), cap)
    return cands[np.arange(n), choice]


def host_prep(cfg, x_b, grid_b, gw_b, eg_b, ega_b, basepts, base_weight, D,
              weights):
    N, NUM_PTS, K = cfg["N"], cfg["NUM_PTS"], cfg["K"]
    CIN, COUT, KM = cfg["CIN"], cfg["COUT"], cfg["KM"]
    G1FIX, ROTS = cfg["G1FIX"], cfg["ROTS"]
    E = K * NUM_PTS
    PCOLS = NUM_PTS // 128
    eg = eg_b.T.reshape(-1).astype(np.int64)        # (E,) [k, p] order
    ega = ega_b.T.reshape(-1).astype(np.int64)
    pp = np.tile(np.arange(NUM_PTS), K)

    # ---------------- xcat pair-row table ----------------
    rows = np.zeros((N, 64), np.float32)
    rows[:, :CIN] = x_b.T
    rows[:, CIN] = grid_b[:, 0]
    rows[:, CIN + 1] = grid_b[:, 1]
    rows[:, CIN + 2] = gw_b
    xcat = rows.astype(np.float16).reshape(N // 2, 128)

    # ---------------- dense tab (rnorm pass) ----------------
    def lay_dense(v):
        return np.ascontiguousarray(
            v.reshape(K, PCOLS, 128).transpose(2, 1, 0).reshape(128, E // 128))
    dtab = np.stack([
        lay_dense(grid_b[eg, 0].reshape(K, NUM_PTS)),
        lay_dense(grid_b[eg, 1].reshape(K, NUM_PTS)),
        lay_dense(gw_b[eg].reshape(K, NUM_PTS)),
        lay_dense(basepts[ega, 0].reshape(K, NUM_PTS)),
        lay_dense(basepts[ega, 1].reshape(K, NUM_PTS)),
    ], axis=-1).astype(np.float16)
    bwd = np.stack([base_weight[:, 0].reshape(PCOLS, 128).T,
                    base_weight[:, 1].reshape(PCOLS, 128).T], axis=-1)

    # ---------------- sigma1: balanced target blocks ----------------
    tdeg = np.bincount(ega, minlength=NUM_PTS)
    blocks1 = _balance_blocks(tdeg, 32, 128, E // 32)
    t_newrow = np.empty(NUM_PTS, np.int64)          # orig target -> new row
    t_local = np.empty(NUM_PTS, np.int64)
    t_block = np.empty(NUM_PTS, np.int64)
    for b in range(32):
        t_newrow[blocks1[b]] = 128 * b + np.arange(128)
        t_local[blocks1[b]] = np.arange(128)
        t_block[blocks1[b]] = b

    SG1 = 32 * G1FIX
    S1 = SG1 * 128
    xidx1 = np.zeros(S1, np.int16)
    tab1 = np.zeros((S1, 8), np.float16)            # bpx bpy bwx bwy me mo egar prow
    tab1[:, 6] = -1.0
    tab1[:, 7] = 127.0                              # no plane match for holes
    for b in range(32):
        sel = np.nonzero(t_block[ega] == b)[0]
        assert len(sel) == E // 32
        res = pp[sel] % 128
        q = _assign_bins(res, ROTS, G1FIX)
        # slot within block: (q, g) with g = rank within bin q
        order = np.argsort(q, kind='stable')
        sel, q = sel[order], q[order]
        cnt = np.bincount(q, minlength=128)
        starts = np.concatenate([[0], np.cumsum(cnt)])[:-1]
        g = np.arange(len(sel)) - starts[q]
        slot = (b * G1FIX + g) * 128 + q
        xidx1[slot] = (eg[sel] >> 1).astype(np.int16)
        tab1[slot, 0] = basepts[ega[sel], 0]
        tab1[slot, 1] = basepts[ega[sel], 1]
        tab1[slot, 2] = base_weight[pp[sel], 0]
        tab1[slot, 3] = base_weight[pp[sel], 1]
        tab1[slot, 4] = (1 - (eg[sel] & 1)).astype(np.float16)
        tab1[slot, 5] = (eg[sel] & 1).astype(np.float16)
        tab1[slot, 6] = t_local[ega[sel]].astype(np.float16)
        rot_used = (q - pp[sel]) % 128
        cidx = np.zeros(len(sel), np.int64)
        for ci, r in enumerate(ROTS):
            cidx[rot_used == r] = ci
        tab1[slot, 7] = (cidx * 32 + (pp[sel] >> 7)).astype(np.float16)

    # tab1 device layout: [128, 8, SG1] (plane-major per partition)
    tab1_dev = np.ascontiguousarray(
        tab1.reshape(SG1, 128, 8).transpose(1, 2, 0)).astype(np.float16)

    # rotation matrices for rnorm planes (f16): R[q, q'] = [q' == (q+rot)%128]
    NROT = len(ROTS) - 1
    rotm = np.zeros((NROT, 128, 128), np.float16)
    for ci, r in enumerate(ROTS[1:]):
        rotm[ci, np.arange(128), (np.arange(128) + r) % 128] = 1.0

    # ---------------- sigma2: balanced pair blocks ----------------
    m2 = eg >> 1
    pdeg = np.bincount(m2, minlength=N // 2)
    blocks2 = _balance_blocks(pdeg, 256, 128, E // 256)
    p_local = np.empty(N // 2, np.int64)
    p_block = np.empty(N // 2, np.int64)
    p_newrow = np.empty(N // 2, np.int64)
    for b in range(256):
        p_local[blocks2[b]] = np.arange(128)
        p_block[blocks2[b]] = b
        p_newrow[blocks2[b]] = 128 * b + np.arange(128)

    SG2 = 1024
    S2 = SG2 * 128
    yidx2 = np.zeros(S2, np.int16)
    tab2 = np.zeros((S2, 8), np.float16)            # gx gy bpx bpy bwx bwy gme gmo... see below
    tab2[:, 7] = -1.0                               # prel hole marker unused (masks=0)
    slot2_of = np.empty(E, np.int64)
    pos = 0
    for b in range(256):
        sel = np.nonzero(p_block[m2] == b)[0]
        assert len(sel) == E // 256
        n = len(sel)
        slot = pos + np.arange(n)
        pos += n
        yidx2[slot] = t_newrow[ega[sel]].astype(np.int16)
        tab2[slot, 0] = grid_b[eg[sel], 0]
        tab2[slot, 1] = grid_b[eg[sel], 1]
        tab2[slot, 2] = basepts[ega[sel], 0]
        tab2[slot, 3] = basepts[ega[sel], 1]
        tab2[slot, 4] = base_weight[pp[sel], 0]
        tab2[slot, 5] = base_weight[pp[sel], 1]
        # plane 6 = prel (pair within block), plane 7 = even-node mask
        tab2[slot, 6] = p_local[m2[sel]].astype(np.float16)
        tab2[slot, 7] = (1 - (eg[sel] & 1)).astype(np.float16)
        slot2_of[sel] = slot
    tab2_dev = np.ascontiguousarray(
        tab2.reshape(SG2, 128, 8).transpose(1, 2, 0)).astype(np.float16)

    # host finish: orig pair row = out_tbl[p_newrow[pair]]

    # dtt rows permuted by target new-row
    t_origin = np.empty(NUM_PTS, np.int64)
    t_origin[t_newrow] = np.arange(NUM_PTS)
    dtt = np.ascontiguousarray(D.T[t_origin].astype(np.float32))

    # tiled iota: [128, G1FIX*128], content[q, g*128+j] = j (one materialized
    # copy per group column so one-hot is_eq needs no stride-0 inner operand)
    iota_row = np.tile(np.arange(128, dtype=np.float16)[None, None, :],
                       (128, G1FIX, 1)).reshape(128, G1FIX * 128)

    return dict(
        xcat=xcat,
        dtab=dtab,
        bwd=np.ascontiguousarray(bwd.astype(np.float32)),
        wfl=np.ascontiguousarray(weights.reshape(CIN, COUT * KM).astype(np.float32)),
        dt_t=dtt,
        xidx1=_wrap16(xidx1),
        tab1=tab1_dev.reshape(128, 8 * SG1),
        rotm=np.ascontiguousarray(rotm.reshape(NROT * 128, 128)),
        yidx2=_wrap16(yidx2),
        tab2=tab2_dev.reshape(128, 8 * SG2),
        iota=np.ascontiguousarray(iota_row),
    ), p_newrow


def build(nc, cfg):
    N, NUM_PTS, K = cfg["N"], cfg["NUM_PTS"], cfg["K"]
    CIN, COUT, KM = cfg["CIN"], cfg["COUT"], cfg["KM"]
    G1FIX = cfg["G1FIX"]
    S2CHUNK = cfg["S2CHUNK"]
    E = K * NUM_PTS
    PCOLS = NUM_PTS // 128
    OJ = COUT * KM
    SG1 = 32 * G1FIX
    SG2 = 1024
    NPLANES = 32 * len(cfg["ROTS"])

    xcat_d = nc.dram_tensor("xcat", [N // 2, 128], f16, kind="ExternalInput")
    dtab_d = nc.dram_tensor("dtab", [128, E // 128, 5], f16, kind="ExternalInput")
    bwd_d = nc.dram_tensor("bwd", [128, PCOLS, 2], f32, kind="ExternalInput")
    wfl_d = nc.dram_tensor("wfl", [CIN, OJ], f32, kind="ExternalInput")
    dtt_d = nc.dram_tensor("dt_t", [NUM_PTS, KM], f32, kind="ExternalInput")
    xidx1_d = nc.dram_tensor("xidx1", [128, SG1 * 128 // 16], i16, kind="ExternalInput")
    tab1_d = nc.dram_tensor("tab1", [128, 8 * SG1], f16, kind="ExternalInput")
    NROT = len(cfg["ROTS"]) - 1
    rotm_d = nc.dram_tensor("rotm", [NROT * 128, 128], f16, kind="ExternalInput")
    yidx2_d = nc.dram_tensor("yidx2", [128, SG2 * 128 // 16], i16, kind="ExternalInput")
    tab2_d = nc.dram_tensor("tab2", [128, 8 * SG2], f16, kind="ExternalInput")
    iota_d = nc.dram_tensor("iota", [128, 128 * cfg["G1FIX"]], f16, kind="ExternalInput")
    out_d = nc.dram_tensor("out", [N // 2, 64], f32, kind="ExternalOutput")
    ycat_d = nc.dram_tensor("ycat_tbl", [NUM_PTS, 64], f32, kind="Internal")

    with tile.TileContext(nc) as tc:
        with tc.tile_pool(name="consts", bufs=1) as cp:
            ident = cp.tile([128, 128], f32)
            masks.make_identity(nc, ident[:])
            nc.gpsimd.load_library(library_config.mlp)

            wfl = cp.tile([CIN, OJ], f32)
            nc.sync.dma_start(wfl[:], wfl_d[:])
            bwd = cp.tile([128, PCOLS * 2], f32)
            bwd3 = bwd[:].rearrange("p (q t) -> p q t", t=2)
            nc.sync.dma_start(bwd3, bwd_d[:])
            iota = cp.tile([128, 128 * G1FIX], f16)
            nc.sync.dma_start(iota[:], iota_d[:])
            rotm = cp.tile([128, NROT * 128], f16)
            nc.sync.dma_start(rotm[:].rearrange("p (c j) -> p c j", c=NROT),
                              rotm_d.ap().rearrange("(c p) j -> p c j", p=128))
            tab1 = cp.tile([128, 8 * SG1], f16)
            nc.sync.dma_start(tab1[:], tab1_d[:])
            tab13 = tab1[:].rearrange("p (t s) -> p t s", t=8)
            xi1 = cp.tile([128, SG1 * 8], i16)
            nc.sync.dma_start(xi1[:], xidx1_d[:])


            # ---------- dense pass: rnorm planes (cp pool: no SBUF reuse
            # so sigma1 gathers are not blocked behind this) ----------
            rnt = cp.tile([128, NPLANES], f16)
            with tc.tile_pool(name="dpsum", bufs=1, space="PSUM") as dq:
                JD = E // 128
                dtab = cp.tile([128, JD * 5], f16)
                dt3 = dtab[:].rearrange("p (j t) -> p j t", t=5)
                nc.sync.dma_start(dt3, dtab_d[:])
                dd0 = cp.tile([128, JD], f32)
                dd1 = cp.tile([128, JD], f32)
                nc.vector.tensor_tensor(dd0[:], dt3[:, :, 0], dt3[:, :, 3], op=subtract)
                nc.vector.tensor_tensor(dd0[:], dd0[:], dd0[:], op=mult)
                nc.vector.tensor_tensor(dd1[:], dt3[:, :, 1], dt3[:, :, 4], op=subtract)
                nc.vector.tensor_tensor(dd1[:], dd1[:], dd1[:], op=mult)
                d0k = dd0[:].rearrange("p (q k) -> p q k", k=K)
                d1k = dd1[:].rearrange("p (q k) -> p q k", k=K)
                nc.vector.tensor_tensor(d0k, d0k,
                                        bwd3[:, :, 0].broadcast_to((128, PCOLS, K)),
                                        op=mult)
                nc.vector.tensor_tensor(d1k, d1k,
                                        bwd3[:, :, 1].broadcast_to((128, PCOLS, K)),
                                        op=mult)
                nc.vector.tensor_tensor(dd0[:], dd0[:], dd1[:], op=add)
                nc.scalar.activation(dd1[:], dd0[:], Exp, scale=-1.0)
                nc.vector.tensor_tensor(dd1[:], dd1[:], dt3[:, :, 2], op=mult)
                nc.vector.tensor_tensor(dd1[:], dd1[:], dd1[:], op=mult)
                nsq = cp.tile([128, PCOLS], f32)
                nc.vector.reduce_sum(nsq[:].unsqueeze(2),
                                     dd1[:].rearrange("p (q k) -> p q k", k=K),
                                     axis=X)
                nc.scalar.activation(nsq[:], nsq[:],
                                     mybir.ActivationFunctionType.Sqrt)
                nc.vector.tensor_scalar_add(nsq[:], nsq[:], 1e-5)
                nc.vector.reciprocal(nsq[:], nsq[:])
                nc.vector.tensor_copy(rnt[:, 0:32], nsq[:])
                for ci in range(NROT):
                    rp = dq.tile([128, 32], f32, tag="rp")
                    nc.tensor.matmul(rp[:], rotm[:, ci * 128:(ci + 1) * 128],
                                     rnt[:, 0:32], start=True, stop=True)
                    nc.vector.tensor_copy(rnt[:, 32 + 32 * ci:64 + 32 * ci], rp[:])

            # ---------- rn_all: 96-plane select (4 chunks so sigma1's early
            # blocks unblock before the whole select finishes) ----------
            rn_all = cp.tile([128, SG1], f16)
            rtmp = cp.tile([128, SG1], f16)
            nc.vector.memset(rn_all[:], 0.0)
            prow = tab13[:, 7, :]
            RC = SG1 // 4
            for r in range(4):
                cs = slice(r * RC, (r + 1) * RC)
                for j in range(NPLANES):
                    nc.vector.scalar_tensor_tensor(
                        rtmp[:, cs], prow[:, cs], float(j),
                        rnt[:, j:j + 1].broadcast_to((128, RC)),
                        op0=is_equal, op1=mult)
                    nc.vector.tensor_tensor(rn_all[:, cs], rn_all[:, cs],
                                            rtmp[:, cs], op=add)

            # ---------- sigma1 + fused phase C ----------
            xhT = cp.tile([CIN, NUM_PTS], f32)
            with tc.tile_pool(name="ph1", bufs=3) as p1, \
                    tc.tile_pool(name="ps1", bufs=2, space="PSUM") as q1:
                for b in range(32):
                    sl = slice(b * G1FIX, (b + 1) * G1FIX)
                    gx = p1.tile([128, G1FIX * 128], f16, tag="gx", bufs=5)
                    gx3 = gx[:].rearrange("p (g e) -> p g e", e=128)
                    nc.gpsimd.dma_gather(
                        gx3, xcat_d[:],
                        xi1[:, b * G1FIX * 8:(b + 1) * G1FIX * 8],
                        G1FIX * 128, G1FIX * 128, 128,
                        elem_step=128, single_packet=False,
                        queue_num=b % 4)
                    me = tab13[:, 4, sl]
                    mo = tab13[:, 5, sl]
                    # grid/gw of the edge's node via parity select
                    ge = p1.tile([128, G1FIX * 3], f32, tag="ge")
                    ge3 = ge[:].rearrange("p (g t) -> p g t", t=3)
                    t0 = p1.tile([128, G1FIX * 3], f32, tag="t0")
                    t03 = t0[:].rearrange("p (g t) -> p g t", t=3)
                    nc.vector.tensor_tensor(
                        ge3, gx3[:, :, 32:35],
                        me.unsqueeze(2).broadcast_to((128, G1FIX, 3)), op=mult)
                    nc.vector.tensor_tensor(
                        t03, gx3[:, :, 96:99],
                        mo.unsqueeze(2).broadcast_to((128, G1FIX, 3)), op=mult)
                    nc.vector.tensor_tensor(ge3, ge3, t03, op=add)
                    dd = p1.tile([128, G1FIX * 2], f32, tag="dd")
                    dd3 = dd[:].rearrange("p (g t) -> p g t", t=2)
                    nc.vector.tensor_tensor(
                        dd3, ge3[:, :, 0:2],
                        tab13[:, 0:2, sl].rearrange("p t s -> p s t"), op=subtract)
                    nc.vector.tensor_tensor(dd3, dd3, dd3, op=mult)
                    nc.vector.tensor_tensor(
                        dd3, dd3,
                        tab13[:, 2:4, sl].rearrange("p t s -> p s t"), op=mult)
                    ga = p1.tile([128, G1FIX], f32, tag="ga")
                    nc.vector.tensor_tensor(ga[:], dd3[:, :, 0], dd3[:, :, 1],
                                            op=add)
                    nc.scalar.activation(ga[:], ga[:], Exp, scale=-1.0)
                    nc.vector.tensor_tensor(ga[:], ga[:], ge3[:, :, 2], op=mult)
                    nc.vector.tensor_tensor(ga[:], ga[:], rn_all[:, sl], op=mult)
                    wlo = p1.tile([128, G1FIX], f32, tag="wlo")
                    whi = p1.tile([128, G1FIX], f32, tag="whi")
                    nc.vector.tensor_tensor(wlo[:], ga[:], me, op=mult)
                    nc.vector.tensor_tensor(whi[:], ga[:], mo, op=mult)
                    v1 = p1.tile([128, G1FIX * CIN], f16, tag="v1")
                    v13 = v1[:].rearrange("p (g e) -> p g e", e=CIN)
                    t1 = p1.tile([128, G1FIX * CIN], f16, tag="t1")
                    t13 = t1[:].rearrange("p (g e) -> p g e", e=CIN)
                    nc.vector.tensor_tensor(
                        v13, gx3[:, :, 0:CIN],
                        wlo[:].unsqueeze(2).broadcast_to((128, G1FIX, CIN)),
                        op=mult)
                    nc.vector.tensor_tensor(
                        t13, gx3[:, :, 64:64 + CIN],
                        whi[:].unsqueeze(2).broadcast_to((128, G1FIX, CIN)),
                        op=mult)
                    nc.vector.tensor_tensor(v13, v13, t13, op=add)
                    oh = p1.tile([128, G1FIX * 128], f16, tag="oh")
                    oh3 = oh[:].rearrange("p (g e) -> p g e", e=128)
                    nc.vector.tensor_tensor(
                        oh3,
                        tab13[:, 6, sl].unsqueeze(2).broadcast_to((128, G1FIX, 128)),
                        iota[:].rearrange("p (g e) -> p g e", e=128),
                        op=is_equal)
                    ps = q1.tile([CIN, 128], f32, tag="pxh")
                    for g in range(G1FIX):
                        nc.tensor.matmul(ps[:], v13[:, g, :], oh3[:, g, :],
                                         start=(g == 0), stop=(g == G1FIX - 1))
                    nc.scalar.activation(xhT[:, b * 128:(b + 1) * 128], ps[:],
                                         mybir.ActivationFunctionType.Copy)
                    # fused phase C for this 128-target tile
                    o1p = q1.tile([128, OJ], f32, tag="o1p")
                    nc.tensor.matmul(o1p[:], xhT[:, b * 128:(b + 1) * 128],
                                     wfl[:], start=True, stop=True)
                    dtt = p1.tile([128, KM], f32, tag="dtt")
                    nc.sync.dma_start(dtt[:], dtt_d[b * 128:(b + 1) * 128, :])
                    o1 = p1.tile([128, OJ], f32, tag="o1")
                    nc.vector.tensor_tensor(
                        o1[:].rearrange("p (o j) -> p o j", j=KM),
                        o1p[:].rearrange("p (o j) -> p o j", j=KM),
                        dtt[:].unsqueeze(1).broadcast_to((128, COUT, KM)),
                        op=mult)
                    yrow = p1.tile([128, 64], f32, tag="yrow")
                    nc.vector.reduce_sum(
                        yrow[:, 0:COUT].unsqueeze(2),
                        o1[:].rearrange("p (o j) -> p o j", j=KM), axis=X)
                    nc.sync.dma_start(
                        ycat_d.ap()[b * 128:(b + 1) * 128, 0:COUT], yrow[:, 0:COUT])

            # ---------- sigma2 ----------
            NCH = 256 // S2CHUNK          # chunks
            GC = S2CHUNK * 4              # group-columns per chunk (G2FIX=4)
            with tc.tile_pool(name="ph2", bufs=2) as p2, \
                    tc.tile_pool(name="ps2", bufs=2, space="PSUM") as q2:
                tab2 = p2.tile([128, 8 * SG2], f16, tag="tab2", bufs=1)
                nc.sync.dma_start(tab2[:], tab2_d[:])
                tab23 = tab2[:].rearrange("p (t s) -> p t s", t=8)
                yi2 = p2.tile([128, SG2 * 8], i16, tag="yi2", bufs=1)
                nc.sync.dma_start(yi2[:], yidx2_d[:])
                for c in range(NCH):
                    s0 = c * GC           # first group-col of chunk
                    sl = slice(s0, s0 + GC)
                    gy = p2.tile([128, GC * 64], f32, tag="gy", bufs=4)
                    gy3 = gy[:].rearrange("p (g e) -> p g e", e=64)
                    nc.gpsimd.dma_gather(
                        gy3, ycat_d[:],
                        yi2[:, s0 * 8:(s0 + GC) * 8],
                        GC * 128, GC * 128, 64,
                        elem_step=64, single_packet=False,
                        queue_num=c % 4)
                    dd = p2.tile([128, GC * 2], f32, tag="dd2")
                    dd3 = dd[:].rearrange("p (g t) -> p g t", t=2)
                    nc.vector.tensor_tensor(
                        dd3, tab23[:, 0:2, sl].rearrange("p t s -> p s t"),
                        tab23[:, 2:4, sl].rearrange("p t s -> p s t"), op=subtract)
                    nc.vector.tensor_tensor(dd3, dd3, dd3, op=mult)
                    nc.vector.tensor_tensor(
                        dd3, dd3,
                        tab23[:, 4:6, sl].rearrange("p t s -> p s t"), op=mult)
                    ga = p2.tile([128, GC], f32, tag="ga2")
                    nc.vector.tensor_tensor(ga[:], dd3[:, :, 0], dd3[:, :, 1],
                                            op=add)
                    nc.scalar.activation(ga[:], ga[:], Exp, scale=-1.0)
                    me2 = tab23[:, 7, sl]
                    gme = p2.tile([128, GC], f32, tag="gme")
                    gmo = p2.tile([128, GC], f32, tag="gmo")
                    nc.vector.tensor_tensor(gme[:], ga[:], me2, op=mult)
                    nc.vector.tensor_tensor(gmo[:], ga[:], gme[:], op=subtract)
                    v2e = p2.tile([128, GC * 32], f16, tag="v2e")
                    v2e3 = v2e[:].rearrange("p (g e) -> p g e", e=32)
                    v2o = p2.tile([128, GC * 32], f16, tag="v2o")
                    v2o3 = v2o[:].rearrange("p (g e) -> p g e", e=32)
                    nc.vector.tensor_tensor(
                        v2e3, gy3[:, :, 0:32],
                        gme[:].unsqueeze(2).broadcast_to((128, GC, 32)), op=mult)
                    nc.vector.tensor_tensor(
                        v2o3, gy3[:, :, 0:32],
                        gmo[:].unsqueeze(2).broadcast_to((128, GC, 32)), op=mult)
                    oh2 = p2.tile([128, GC * 128], f16, tag="oh2")
                    oh23 = oh2[:].rearrange("p (g e) -> p g e", e=128)
                    nc.vector.tensor_tensor(
                        oh23,
                        tab23[:, 6, sl].unsqueeze(2).broadcast_to((128, GC, 128)),
                        iota[:, :GC * 128].rearrange("p (g e) -> p g e", e=128),
                        op=is_equal)
                    ob = p2.tile([128, S2CHUNK * 64], f32, tag="ob")
                    ob3 = ob[:].rearrange("p (k e) -> p k e", e=64)
                    for k in range(S2CHUNK):
                        po = q2.tile([128, 64], f32, tag="po")
                        po3 = po[:].rearrange("p (h e) -> p h e", e=32)
                        for g in range(4):
                            gc = 4 * k + g
                            nc.tensor.matmul(po3[:, 0, :], oh23[:, gc, :],
                                             v2e3[:, gc, :],
                                             start=(g == 0), stop=(g == 3))
                        for g in range(4):
                            gc = 4 * k + g
                            nc.tensor.matmul(po3[:, 1, :], oh23[:, gc, :],
                                             v2o3[:, gc, :],
                                             start=(g == 0), stop=(g == 3))
                        nc.scalar.activation(ob3[:, k, :], po[:],
                                             mybir.ActivationFunctionType.Copy)
                    nc.sync.dma_start(
                        out_d.ap()[c * S2CHUNK * 128:(c + 1) * S2CHUNK * 128, :]
                        .rearrange("(k p) e -> p k e", p=128),
                        ob3)
    return nc


def make_in_maps(cfg, x, grid, grid_weight, edge_grid, edge_Gauss, basepts,
                 base_weight, D, weights):
    maps, invs = [], []
    for b in range(x.shape[0]):
        m, inv2 = host_prep(cfg, x[b], grid[b], grid_weight[b], edge_grid[b],
                            edge_Gauss[b], basepts, base_weight, D, weights)
        maps.append(m)
        invs.append(inv2)
    return maps, invs


def finish(cfg, out_tbl, p_newrow):
    # device row p_newrow[pair] holds pair's output
    o = out_tbl[p_newrow]
    return np.ascontiguousarray(
        o.reshape(cfg["N"], 32)[:, :cfg["COUT"]].T)


_BUILT = {}


def _get_nc():
    if "nc" not in _BUILT:
        nc = bacc.Bacc("TRN2", target_bir_lowering=False,
                       dynamic_dma_scratch_size=32768,
                       num_swdge_queues=4)
        build(nc, CFG)
        nc.compile()
        _BUILT["nc"] = nc
    return _BUILT["nc"]


def kernel(x, grid, grid_weight, edge_grid, edge_Gauss, basepts, base_weight,
           D, weights, _trace=False):
    cfg = CFG
    in_maps, invs = make_in_maps(
        cfg, np.asarray(x, np.float32), np.asarray(grid),
        np.asarray(grid_weight), np.asarray(edge_grid),
        np.asarray(edge_Gauss), np.asarray(basepts),
        np.asarray(base_weight), np.asarray(D), np.asarray(weights))
    nc = _get_nc()
    res = bass_utils.run_bass_kernel_spmd(
        nc, in_maps, core_ids=list(range(x.shape[0])), trace=_trace)
    out = np.stack([finish(cfg, res.results[b]["out"], invs[b])
                    for b in range(x.shape[0])])
    kernel.last_result = res
    return out


# revision 25
# speedup vs baseline: 1.2763x; 1.0921x over previous
"""GPDconv (GNN message passing) Trainium2 Bass kernel — PE one-hot design.

Batch-parallel over 8 NeuronCores (one batch per core). The previous design
spent ~4ms/core in Q7 SWDGE descriptor generation (~8ns per gather index,
~500k indices). This version keeps exactly TWO per-edge SWDGE passes (the
provable floor) and does all aggregation on the PE via one-hot matmuls:

  sigma1: edges sorted into 32 host-balanced target-blocks (128 ega-targets,
    exactly 4096 edges each). One dma_gather of x pair-rows per edge
    (+ ~6% slot padding from the rnorm partition constraint). Per 128-edge
    group: V1 = u*rnorm*x_row, one-hot over within-block target -> PE matmul
    accumulating x_hat^T [32ch, 128t] in PSUM. rnorm[p] is delivered by a
    96-plane select: edge partition q == (p + rot_c) % 128 for one of three
    rotations (3-choice load balancing), rnorm planes live at [q, 32c+j].
  C: y = (x_hat @ W) . D^T per 128-target tile (targets in permuted order).
  sigma2: edges sorted into 256 host-balanced pair-blocks (128 node-pairs,
    exactly 512 edges each). One dma_gather of y rows per edge (zero pad).
    V2 = gauss*(parity masks)*y, one-hot over within-block pair -> PE matmul
    -> out pair-rows [128, 64] per block, written permuted; host unpermutes.

Host does index/layout prep only (sorting, balancing, packing, int16);
all value math (gauss, norms, products, reductions) runs on device.
"""
import sys

if '/opt/trn_rl_repo' not in sys.path:
    sys.path.insert(0, '/opt/trn_rl_repo')

import numpy as np
import concourse.bacc as bacc
import concourse.mybir as mybir
import concourse.tile as tile
from concourse import bass_utils, library_config, masks

f32 = mybir.dt.float32
f16 = mybir.dt.float16
i16 = mybir.dt.int16

CFG = dict(N=65536, NUM_PTS=4096, K=32, CIN=32, COUT=32, KM=16,
           G1FIX=34, ROTS=(0, 43, 86), S2CHUNK=8)

mult, add, subtract = (mybir.AluOpType.mult, mybir.AluOpType.add,
                       mybir.AluOpType.subtract)
is_equal = mybir.AluOpType.is_equal
Exp = mybir.ActivationFunctionType.Exp
X = mybir.AxisListType.X


def _wrap16(a):
    return np.ascontiguousarray(np.tile(a.reshape(-1, 16).T, (8, 1)))


def _balance_blocks(deg, nblocks, per_block_items, per_block_sum):
    """Partition items into nblocks of exactly per_block_items items with
    degree sums exactly per_block_sum. Snake-deal + exact swap repair."""
    deg = np.asarray(deg, np.int64)
    n = len(deg)
    assert n == nblocks * per_block_items
    assert deg.sum() == nblocks * per_block_sum
    order = np.argsort(-deg, kind='stable')
    # snake deal: rows of nblocks, alternating direction
    rows = order.reshape(per_block_items, nblocks)
    for r in range(1, per_block_items, 2):
        rows[r] = rows[r][::-1]
    blocks = [list(rows[:, b]) for b in range(nblocks)]
    sums = np.array([deg[b].sum() for b in blocks], np.int64)
    for _ in range(100000):
        dev = sums - per_block_sum
        if not dev.any():
            break
        hi = int(np.argmax(dev))
        lo = int(np.argmin(dev))
        dstar = int(min(dev[hi], -dev[lo]))
        ha = np.asarray(blocks[hi])
        la = np.asarray(blocks[lo])
        da, db = deg[ha], deg[la]
        ua = np.unique(da)
        ub = np.unique(db)
        found = None
        for want in range(dstar, 0, -1):
            hit = ua[np.isin(ua - want, ub)]
            if len(hit):
                va = int(hit[0])
                ai = int(np.nonzero(da == va)[0][0])
                bj = int(np.nonzero(db == va - want)[0][0])
                found = (ai, bj, want)
                break
        assert found is not None, (dev[hi], dev[lo], ua, ub)
        ai, bj, want = found
        a_it, b_it = int(ha[ai]), int(la[bj])
        blocks[hi][ai] = b_it
        blocks[lo][bj] = a_it
        sums[hi] -= want
        sums[lo] += want
    assert (sums == per_block_sum).all(), sums
    return [np.asarray(b, np.int64) for b in blocks]


def _assign_bins(res, rots, cap):
    """3-choice capacitated assignment: edge i may go to bin
    (res[i]+rot)%128; return bin per edge with loads <= cap.
    Greedy lightest-bin init + BFS augmenting-path eviction."""
    n = len(res)
    nr = len(rots)
    cands = np.stack([(res + r) % 128 for r in rots], 1)   # (n, nr)
    cnt = np.zeros(128, np.int64)
    choice = np.zeros(n, np.int64)
    order = np.random.default_rng(0).permutation(n)
    for i in order:
        c = cands[i]
        j = int(np.argmin(cnt[c]))
        choice[i] = j
        cnt[c[j]] += 1
    # bin -> member edge list
    members = [[] for _ in range(128)]
    for i in range(n):
        members[int(cands[i, choice[i]])].append(i)
    while True:
        over = [b for b in range(128) if cnt[b] > cap]
        if not over:
            break
        s = over[0]
        # BFS from s to any bin with load < cap via edge reassignments
        parent = {s: None}
        frontier = [s]
        goal = None
        while frontier and goal is None:
            nxt = []
            for u in frontier:
                for i in members[u]:
                    for j in range(nr):
                        v = int(cands[i, j])
                        if v == u or v in parent:
                            continue
                        parent[v] = (u, i, j)
                        if cnt[v] < cap:
                            goal = v
                            break
                        nxt.append(v)
                    if goal is not None:
                        break
                if goal is not None:
                    break
            frontier = nxt
        assert goal is not None, "no augmenting path; raise G1FIX"
        # walk back, reassigning one edge per hop
        v = goal
        while parent[v] is not None:
            u, i, j = parent[v]
            members[u].remove(i)
            members[v].append(i)
            choice[i] = j
            cnt[u] -= 1
            cnt[v] += 1
            v = u
    assert cnt.max() <= cap, (cnt.max(), cap)
    return cands[np.arange(n), choice]


def host_prep(cfg, x_b, grid_b, gw_b, eg_b, ega_b, basepts, base_weight, D,
              weights):
    N, NUM_PTS, K = cfg["N"], cfg["NUM_PTS"], cfg["K"]
    CIN, COUT, KM = cfg["CIN"], cfg["COUT"], cfg["KM"]
    G1FIX, ROTS = cfg["G1FIX"], cfg["ROTS"]
    E = K * NUM_PTS
    PCOLS = NUM_PTS // 128
    eg = eg_b.T.reshape(-1).astype(np.int64)        # (E,) [k, p] order
    ega = ega_b.T.reshape(-1).astype(np.int64)
    pp = np.tile(np.arange(NUM_PTS), K)

    # ---------------- xcat pair-row table ----------------
    rows = np.zeros((N, 64), np.float32)
    rows[:, :CIN] = x_b.T
    rows[:, CIN] = grid_b[:, 0]
    rows[:, CIN + 1] = grid_b[:, 1]
    rows[:, CIN + 2] = gw_b
    xcat = rows.astype(np.float16).reshape(N // 2, 128)

    # ---------------- dense tab (rnorm pass) ----------------
    def lay_dense(v):
        return np.ascontiguousarray(
            v.reshape(K, PCOLS, 128).transpose(2, 1, 0).reshape(128, E // 128))
    dtab = np.stack([
        lay_dense(grid_b[eg, 0].reshape(K, NUM_PTS)),
        lay_dense(grid_b[eg, 1].reshape(K, NUM_PTS)),
        lay_dense(gw_b[eg].reshape(K, NUM_PTS)),
        lay_dense(basepts[ega, 0].reshape(K, NUM_PTS)),
        lay_dense(basepts[ega, 1].reshape(K, NUM_PTS)),
    ], axis=-1).astype(np.float16)
    bwd = np.stack([base_weight[:, 0].reshape(PCOLS, 128).T,
                    base_weight[:, 1].reshape(PCOLS, 128).T], axis=-1)

    # ---------------- sigma1: balanced target blocks ----------------
    tdeg = np.bincount(ega, minlength=NUM_PTS)
    blocks1 = _balance_blocks(tdeg, 32, 128, E // 32)
    t_newrow = np.empty(NUM_PTS, np.int64)          # orig target -> new row
    t_local = np.empty(NUM_PTS, np.int64)
    t_block = np.empty(NUM_PTS, np.int64)
    for b in range(32):
        t_newrow[blocks1[b]] = 128 * b + np.arange(128)
        t_local[blocks1[b]] = np.arange(128)
        t_block[blocks1[b]] = b

    SG1 = 32 * G1FIX
    S1 = SG1 * 128
    xidx1 = np.zeros(S1, np.int16)
    tab1 = np.zeros((S1, 8), np.float16)            # bpx bpy bwx bwy me mo egar prow
    tab1[:, 6] = -1.0
    tab1[:, 7] = 127.0                              # no plane match for holes
    for b in range(32):
        sel = np.nonzero(t_block[ega] == b)[0]
        assert len(sel) == E // 32
        res = pp[sel] % 128
        q = _assign_bins(res, ROTS, G1FIX)
        # slot within block: (q, g) with g = rank within bin q
        order = np.argsort(q, kind='stable')
        sel, q = sel[order], q[order]
        cnt = np.bincount(q, minlength=128)
        starts = np.concatenate([[0], np.cumsum(cnt)])[:-1]
        g = np.arange(len(sel)) - starts[q]
        slot = (b * G1FIX + g) * 128 + q
        xidx1[slot] = (eg[sel] >> 1).astype(np.int16)
        tab1[slot, 0] = basepts[ega[sel], 0]
        tab1[slot, 1] = basepts[ega[sel], 1]
        tab1[slot, 2] = base_weight[pp[sel], 0]
        tab1[slot, 3] = base_weight[pp[sel], 1]
        tab1[slot, 4] = (1 - (eg[sel] & 1)).astype(np.float16)
        tab1[slot, 5] = (eg[sel] & 1).astype(np.float16)
        tab1[slot, 6] = t_local[ega[sel]].astype(np.float16)
        rot_used = (q - pp[sel]) % 128
        cidx = np.zeros(len(sel), np.int64)
        for ci, r in enumerate(ROTS):
            cidx[rot_used == r] = ci
        tab1[slot, 7] = (cidx * 32 + (pp[sel] >> 7)).astype(np.float16)

    # tab1 device layout: [128, 8, SG1] (plane-major per partition)
    tab1_dev = np.ascontiguousarray(
        tab1.reshape(SG1, 128, 8).transpose(1, 2, 0)).astype(np.float16)

    # rotation matrices for rnorm planes (f16): R[q, q'] = [q' == (q+rot)%128]
    NROT = len(ROTS) - 1
    rotm = np.zeros((NROT, 128, 128), np.float16)
    for ci, r in enumerate(ROTS[1:]):
        rotm[ci, np.arange(128), (np.arange(128) + r) % 128] = 1.0

    # ---------------- sigma2: balanced pair blocks ----------------
    m2 = eg >> 1
    pdeg = np.bincount(m2, minlength=N // 2)
    blocks2 = _balance_blocks(pdeg, 256, 128, E // 256)
    p_local = np.empty(N // 2, np.int64)
    p_block = np.empty(N // 2, np.int64)
    p_newrow = np.empty(N // 2, np.int64)
    for b in range(256):
        p_local[blocks2[b]] = np.arange(128)
        p_block[blocks2[b]] = b
        p_newrow[blocks2[b]] = 128 * b + np.arange(128)

    SG2 = 1024
    S2 = SG2 * 128
    yidx2 = np.zeros(S2, np.int16)
    tab2 = np.zeros((S2, 8), np.float16)            # gx gy bpx bpy bwx bwy gme gmo... see below
    tab2[:, 7] = -1.0                               # prel hole marker unused (masks=0)
    slot2_of = np.empty(E, np.int64)
    pos = 0
    for b in range(256):
        sel = np.nonzero(p_block[m2] == b)[0]
        assert len(sel) == E // 256
        n = len(sel)
        slot = pos + np.arange(n)
        pos += n
        yidx2[slot] = t_newrow[ega[sel]].astype(np.int16)
        tab2[slot, 0] = grid_b[eg[sel], 0]
        tab2[slot, 1] = grid_b[eg[sel], 1]
        tab2[slot, 2] = basepts[ega[sel], 0]
        tab2[slot, 3] = basepts[ega[sel], 1]
        tab2[slot, 4] = base_weight[pp[sel], 0]
        tab2[slot, 5] = base_weight[pp[sel], 1]
        # plane 6 = prel (pair within block), plane 7 = even-node mask
        tab2[slot, 6] = p_local[m2[sel]].astype(np.float16)
        tab2[slot, 7] = (1 - (eg[sel] & 1)).astype(np.float16)
        slot2_of[sel] = slot
    tab2_dev = np.ascontiguousarray(
        tab2.reshape(SG2, 128, 8).transpose(1, 2, 0)).astype(np.float16)

    # host finish: orig pair row = out_tbl[p_newrow[pair]]

    # dtt rows permuted by target new-row
    t_origin = np.empty(NUM_PTS, np.int64)
    t_origin[t_newrow] = np.arange(NUM_PTS)
    dtt = np.ascontiguousarray(D.T[t_origin].astype(np.float32))

    # tiled iota: [128, G1FIX*128], content[q, g*128+j] = j (one materialized
    # copy per group column so one-hot is_eq needs no stride-0 inner operand)
    iota_row = np.tile(np.arange(128, dtype=np.float16)[None, None, :],
                       (128, G1FIX, 1)).reshape(128, G1FIX * 128)

    return dict(
        xcat=xcat,
        dtab=dtab,
        bwd=np.ascontiguousarray(bwd.astype(np.float32)),
        wfl=np.ascontiguousarray(weights.reshape(CIN, COUT * KM).astype(np.float32)),
        dt_t=dtt,
        xidx1=_wrap16(xidx1),
        tab1=tab1_dev.reshape(128, 8 * SG1),
        rotm=np.ascontiguousarray(rotm.reshape(NROT * 128, 128)),
        yidx2=_wrap16(yidx2),
        tab2=tab2_dev.reshape(128, 8 * SG2),
        iota=np.ascontiguousarray(iota_row),
    ), p_newrow


def build(nc, cfg):
    N, NUM_PTS, K = cfg["N"], cfg["NUM_PTS"], cfg["K"]
    CIN, COUT, KM = cfg["CIN"], cfg["COUT"], cfg["KM"]
    G1FIX = cfg["G1FIX"]
    S2CHUNK = cfg["S2CHUNK"]
    E = K * NUM_PTS
    PCOLS = NUM_PTS // 128
    OJ = COUT * KM
    SG1 = 32 * G1FIX
    SG2 = 1024
    NPLANES = 32 * len(cfg["ROTS"])

    xcat_d = nc.dram_tensor("xcat", [N // 2, 128], f16, kind="ExternalInput")
    dtab_d = nc.dram_tensor("dtab", [128, E // 128, 5], f16, kind="ExternalInput")
    bwd_d = nc.dram_tensor("bwd", [128, PCOLS, 2], f32, kind="ExternalInput")
    wfl_d = nc.dram_tensor("wfl", [CIN, OJ], f32, kind="ExternalInput")
    dtt_d = nc.dram_tensor("dt_t", [NUM_PTS, KM], f32, kind="ExternalInput")
    xidx1_d = nc.dram_tensor("xidx1", [128, SG1 * 128 // 16], i16, kind="ExternalInput")
    tab1_d = nc.dram_tensor("tab1", [128, 8 * SG1], f16, kind="ExternalInput")
    NROT = len(cfg["ROTS"]) - 1
    rotm_d = nc.dram_tensor("rotm", [NROT * 128, 128], f16, kind="ExternalInput")
    yidx2_d = nc.dram_tensor("yidx2", [128, SG2 * 128 // 16], i16, kind="ExternalInput")
    tab2_d = nc.dram_tensor("tab2", [128, 8 * SG2], f16, kind="ExternalInput")
    iota_d = nc.dram_tensor("iota", [128, 128 * cfg["G1FIX"]], f16, kind="ExternalInput")
    out_d = nc.dram_tensor("out", [N // 2, 64], f32, kind="ExternalOutput")
    ycat_d = nc.dram_tensor("ycat_tbl", [NUM_PTS, 64], f32, kind="Internal")

    with tile.TileContext(nc) as tc:
        with tc.tile_pool(name="consts", bufs=1) as cp:
            ident = cp.tile([128, 128], f32)
            masks.make_identity(nc, ident[:])
            nc.gpsimd.load_library(library_config.mlp)

            wfl = cp.tile([CIN, OJ], f32)
            nc.sync.dma_start(wfl[:], wfl_d[:])
            bwd = cp.tile([128, PCOLS * 2], f32)
            bwd3 = bwd[:].rearrange("p (q t) -> p q t", t=2)
            nc.sync.dma_start(bwd3, bwd_d[:])
            iota = cp.tile([128, 128 * G1FIX], f16)
            nc.sync.dma_start(iota[:], iota_d[:])
            rotm = cp.tile([128, NROT * 128], f16)
            nc.sync.dma_start(rotm[:].rearrange("p (c j) -> p c j", c=NROT),
                              rotm_d.ap().rearrange("(c p) j -> p c j", p=128))
            tab1 = cp.tile([128, 8 * SG1], f16)
            nc.sync.dma_start(tab1[:], tab1_d[:])
            tab13 = tab1[:].rearrange("p (t s) -> p t s", t=8)
            xi1 = cp.tile([128, SG1 * 8], i16)
            nc.sync.dma_start(xi1[:], xidx1_d[:])


            # ---------- dense pass: rnorm planes (cp pool: no SBUF reuse
            # so sigma1 gathers are not blocked behind this) ----------
            rnt = cp.tile([128, NPLANES], f16)
            with tc.tile_pool(name="dpsum", bufs=1, space="PSUM") as dq:
                JD = E // 128
                dtab = cp.tile([128, JD * 5], f16)
                dt3 = dtab[:].rearrange("p (j t) -> p j t", t=5)
                nc.sync.dma_start(dt3, dtab_d[:])
                dd0 = cp.tile([128, JD], f32)
                dd1 = cp.tile([128, JD], f32)
                nc.vector.tensor_tensor(dd0[:], dt3[:, :, 0], dt3[:, :, 3], op=subtract)
                nc.vector.tensor_tensor(dd0[:], dd0[:], dd0[:], op=mult)
                nc.vector.tensor_tensor(dd1[:], dt3[:, :, 1], dt3[:, :, 4], op=subtract)
                nc.vector.tensor_tensor(dd1[:], dd1[:], dd1[:], op=mult)
                d0k = dd0[:].rearrange("p (q k) -> p q k", k=K)
                d1k = dd1[:].rearrange("p (q k) -> p q k", k=K)
                nc.vector.tensor_tensor(d0k, d0k,
                                        bwd3[:, :, 0].broadcast_to((128, PCOLS, K)),
                                        op=mult)
                nc.vector.tensor_tensor(d1k, d1k,
                                        bwd3[:, :, 1].broadcast_to((128, PCOLS, K)),
                                        op=mult)
                nc.vector.tensor_tensor(dd0[:], dd0[:], dd1[:], op=add)
                nc.scalar.activation(dd1[:], dd0[:], Exp, scale=-1.0)
                nc.vector.tensor_tensor(dd1[:], dd1[:], dt3[:, :, 2], op=mult)
                nc.vector.tensor_tensor(dd1[:], dd1[:], dd1[:], op=mult)
                nsq = cp.tile([128, PCOLS], f32)
                nc.vector.reduce_sum(nsq[:].unsqueeze(2),
                                     dd1[:].rearrange("p (q k) -> p q k", k=K),
                                     axis=X)
                nc.scalar.activation(nsq[:], nsq[:],
                                     mybir.ActivationFunctionType.Sqrt)
                nc.vector.tensor_scalar_add(nsq[:], nsq[:], 1e-5)
                nc.vector.reciprocal(nsq[:], nsq[:])
                nc.vector.tensor_copy(rnt[:, 0:32], nsq[:])
                for ci in range(NROT):
                    rp = dq.tile([128, 32], f32, tag="rp")
                    nc.tensor.matmul(rp[:], rotm[:, ci * 128:(ci + 1) * 128],
                                     rnt[:, 0:32], start=True, stop=True)
                    nc.vector.tensor_copy(rnt[:, 32 + 32 * ci:64 + 32 * ci], rp[:])

            # ---------- rn_all: 96-plane select (4 chunks so sigma1's early
            # blocks unblock before the whole select finishes) ----------
            rn_all = cp.tile([128, SG1], f16)
            rtmp = cp.tile([128, SG1], f16)
            nc.vector.memset(rn_all[:], 0.0)
            prow = tab13[:, 7, :]
            RC = SG1 // 4
            for r in range(4):
                cs = slice(r * RC, (r + 1) * RC)
                for j in range(NPLANES):
                    nc.vector.scalar_tensor_tensor(
                        rtmp[:, cs], prow[:, cs], float(j),
                        rnt[:, j:j + 1].broadcast_to((128, RC)),
                        op0=is_equal, op1=mult)
                    nc.vector.tensor_tensor(rn_all[:, cs], rn_all[:, cs],
                                            rtmp[:, cs], op=add)

            # ---------- sigma1 + fused phase C ----------
            xhT = cp.tile([CIN, NUM_PTS], f32)
            with tc.tile_pool(name="ph1", bufs=3) as p1, \
                    tc.tile_pool(name="ps1", bufs=2, space="PSUM") as q1:
                for b in range(32):
                    sl = slice(b * G1FIX, (b + 1) * G1FIX)
                    gx = p1.tile([128, G1FIX * 128], f16, tag="gx", bufs=5)
                    gx3 = gx[:].rearrange("p (g e) -> p g e", e=128)
                    nc.gpsimd.dma_gather(
                        gx3, xcat_d[:],
                        xi1[:, b * G1FIX * 8:(b + 1) * G1FIX * 8],
                        G1FIX * 128, G1FIX * 128, 128,
                        elem_step=128, single_packet=False,
                        queue_num=b % 4)
                    me = tab13[:, 4, sl]
                    mo = tab13[:, 5, sl]
                    # grid/gw of the edge's node via parity select
                    ge = p1.tile([128, G1FIX * 3], f32, tag="ge")
                    ge3 = ge[:].rearrange("p (g t) -> p g t", t=3)
                    t0 = p1.tile([128, G1FIX * 3], f32, tag="t0")
                    t03 = t0[:].rearrange("p (g t) -> p g t", t=3)
                    nc.vector.tensor_tensor(
                        ge3, gx3[:, :, 32:35],
                        me.unsqueeze(2).broadcast_to((128, G1FIX, 3)), op=mult)
                    nc.vector.tensor_tensor(
                        t03, gx3[:, :, 96:99],
                        mo.unsqueeze(2).broadcast_to((128, G1FIX, 3)), op=mult)
                    nc.vector.tensor_tensor(ge3, ge3, t03, op=add)
                    dd = p1.tile([128, G1FIX * 2], f32, tag="dd")
                    dd3 = dd[:].rearrange("p (g t) -> p g t", t=2)
                    nc.vector.tensor_tensor(
                        dd3, ge3[:, :, 0:2],
                        tab13[:, 0:2, sl].rearrange("p t s -> p s t"), op=subtract)
                    nc.vector.tensor_tensor(dd3, dd3, dd3, op=mult)
                    nc.vector.tensor_tensor(
                        dd3, dd3,
                        tab13[:, 2:4, sl].rearrange("p t s -> p s t"), op=mult)
                    ga = p1.tile([128, G1FIX], f32, tag="ga")
                    nc.vector.tensor_tensor(ga[:], dd3[:, :, 0], dd3[:, :, 1],
                                            op=add)
                    nc.scalar.activation(ga[:], ga[:], Exp, scale=-1.0)
                    nc.vector.tensor_tensor(ga[:], ga[:], ge3[:, :, 2], op=mult)
                    nc.vector.tensor_tensor(ga[:], ga[:], rn_all[:, sl], op=mult)
                    wlo = p1.tile([128, G1FIX], f32, tag="wlo")
                    whi = p1.tile([128, G1FIX], f32, tag="whi")
                    nc.vector.tensor_tensor(wlo[:], ga[:], me, op=mult)
                    nc.vector.tensor_tensor(whi[:], ga[:], mo, op=mult)
                    v1 = p1.tile([128, G1FIX * CIN], f16, tag="v1")
                    v13 = v1[:].rearrange("p (g e) -> p g e", e=CIN)
                    t1 = p1.tile([128, G1FIX * CIN], f16, tag="t1")
                    t13 = t1[:].rearrange("p (g e) -> p g e", e=CIN)
                    nc.vector.tensor_tensor(
                        v13, gx3[:, :, 0:CIN],
                        wlo[:].unsqueeze(2).broadcast_to((128, G1FIX, CIN)),
                        op=mult)
                    nc.vector.tensor_tensor(
                        t13, gx3[:, :, 64:64 + CIN],
                        whi[:].unsqueeze(2).broadcast_to((128, G1FIX, CIN)),
                        op=mult)
                    nc.vector.tensor_tensor(v13, v13, t13, op=add)
                    oh = p1.tile([128, G1FIX * 128], f16, tag="oh")
                    oh3 = oh[:].rearrange("p (g e) -> p g e", e=128)
                    nc.vector.tensor_tensor(
                        oh3,
                        tab13[:, 6, sl].unsqueeze(2).broadcast_to((128, G1FIX, 128)),
                        iota[:].rearrange("p (g e) -> p g e", e=128),
                        op=is_equal)
                    ps = q1.tile([CIN, 128], f32, tag="pxh")
                    for g in range(G1FIX):
                        nc.tensor.matmul(ps[:], v13[:, g, :], oh3[:, g, :],
                                         start=(g == 0), stop=(g == G1FIX - 1))
                    nc.scalar.activation(xhT[:, b * 128:(b + 1) * 128], ps[:],
                                         mybir.ActivationFunctionType.Copy)
                    # fused phase C for this 128-target tile
                    o1p = q1.tile([128, OJ], f32, tag="o1p")
                    nc.tensor.matmul(o1p[:], xhT[:, b * 128:(b + 1) * 128],
                                     wfl[:], start=True, stop=True)
                    dtt = p1.tile([128, KM], f32, tag="dtt")
                    nc.sync.dma_start(dtt[:], dtt_d[b * 128:(b + 1) * 128, :])
                    o1 = p1.tile([128, OJ], f32, tag="o1")
                    nc.vector.tensor_tensor(
                        o1[:].rearrange("p (o j) -> p o j", j=KM),
                        o1p[:].rearrange("p (o j) -> p o j", j=KM),
                        dtt[:].unsqueeze(1).broadcast_to((128, COUT, KM)),
                        op=mult)
                    yrow = p1.tile([128, 64], f32, tag="yrow")
                    nc.vector.reduce_sum(
                        yrow[:, 0:COUT].unsqueeze(2),
                        o1[:].rearrange("p (o j) -> p o j", j=KM), axis=X)
                    nc.sync.dma_start(
                        ycat_d.ap()[b * 128:(b + 1) * 128, 0:COUT], yrow[:, 0:COUT])

            # ---------- sigma2 ----------
            NCH = 256 // S2CHUNK          # chunks
            GC = S2CHUNK * 4              # group-columns per chunk (G2FIX=4)
            with tc.tile_pool(name="ph2", bufs=2) as p2, \
                    tc.tile_pool(name="ps2", bufs=2, space="PSUM") as q2:
                tab2 = p2.tile([128, 8 * SG2], f16, tag="tab2", bufs=1)
                nc.sync.dma_start(tab2[:], tab2_d[:])
                tab23 = tab2[:].rearrange("p (t s) -> p t s", t=8)
                yi2 = p2.tile([128, SG2 * 8], i16, tag="yi2", bufs=1)
                nc.sync.dma_start(yi2[:], yidx2_d[:])
                for c in range(NCH):
                    s0 = c * GC           # first group-col of chunk
                    sl = slice(s0, s0 + GC)
                    gy = p2.tile([128, GC * 64], f32, tag="gy", bufs=4)
                    gy3 = gy[:].rearrange("p (g e) -> p g e", e=64)
                    nc.gpsimd.dma_gather(
                        gy3, ycat_d[:],
                        yi2[:, s0 * 8:(s0 + GC) * 8],
                        GC * 128, GC * 128, 64,
                        elem_step=64, single_packet=False,
                        queue_num=c % 4)
                    dd = p2.tile([128, GC * 2], f32, tag="dd2")
                    dd3 = dd[:].rearrange("p (g t) -> p g t", t=2)
                    nc.vector.tensor_tensor(
                        dd3, tab23[:, 0:2, sl].rearrange("p t s -> p s t"),
                        tab23[:, 2:4, sl].rearrange("p t s -> p s t"), op=subtract)
                    nc.vector.tensor_tensor(dd3, dd3, dd3, op=mult)
                    nc.vector.tensor_tensor(
                        dd3, dd3,
                        tab23[:, 4:6, sl].rearrange("p t s -> p s t"), op=mult)
                    ga = p2.tile([128, GC], f32, tag="ga2")
                    nc.vector.tensor_tensor(ga[:], dd3[:, :, 0], dd3[:, :, 1],
                                            op=add)
                    nc.scalar.activation(ga[:], ga[:], Exp, scale=-1.0)
                    me2 = tab23[:, 7, sl]
                    gme = p2.tile([128, GC], f32, tag="gme")
                    gmo = p2.tile([128, GC], f32, tag="gmo")
                    nc.vector.tensor_tensor(gme[:], ga[:], me2, op=mult)
                    nc.vector.tensor_tensor(gmo[:], ga[:], gme[:], op=subtract)
                    v2e = p2.tile([128, GC * 32], f16, tag="v2e")
                    v2e3 = v2e[:].rearrange("p (g e) -> p g e", e=32)
                    v2o = p2.tile([128, GC * 32], f16, tag="v2o")
                    v2o3 = v2o[:].rearrange("p (g e) -> p g e", e=32)
                    nc.vector.tensor_tensor(
                        v2e3, gy3[:, :, 0:32],
                        gme[:].unsqueeze(2).broadcast_to((128, GC, 32)), op=mult)
                    nc.vector.tensor_tensor(
                        v2o3, gy3[:, :, 0:32],
                        gmo[:].unsqueeze(2).broadcast_to((128, GC, 32)), op=mult)
                    oh2 = p2.tile([128, GC * 128], f16, tag="oh2")
                    oh23 = oh2[:].rearrange("p (g e) -> p g e", e=128)
                    nc.vector.tensor_tensor(
                        oh23,
                        tab23[:, 6, sl].unsqueeze(2).broadcast_to((128, GC, 128)),
                        iota[:, :GC * 128].rearrange("p (g e) -> p g e", e=128),
                        op=is_equal)
                    ob = p2.tile([128, S2CHUNK * 64], f32, tag="ob")
                    ob3 = ob[:].rearrange("p (k e) -> p k e", e=64)
                    for k in range(S2CHUNK):
                        po = q2.tile([128, 64], f32, tag="po")
                        po3 = po[:].rearrange("p (h e) -> p h e", e=32)
                        for g in range(4):
                            gc = 4 * k + g
                            nc.tensor.matmul(po3[:, 0, :], oh23[:, gc, :],
                                             v2e3[:, gc, :],
                                             start=(g == 0), stop=(g == 3))
                        for g in range(4):
                            gc = 4 * k + g
                            nc.tensor.matmul(po3[:, 1, :], oh23[:, gc, :],
                                             v2o3[:, gc, :],
                                             start=(g == 0), stop=(g == 3))
                        nc.scalar.activation(ob3[:, k, :], po[:],
                                             mybir.ActivationFunctionType.Copy)
                    nc.sync.dma_start(
                        out_d.ap()[c * S2CHUNK * 128:(c + 1) * S2CHUNK * 128, :]
                        .rearrange("(k p) e -> p k e", p=128),
                        ob3)
    return nc


def make_in_maps(cfg, x, grid, grid_weight, edge_grid, edge_Gauss, basepts,
                 base_weight, D, weights):
    maps, invs = [], []
    for b in range(x.shape[0]):
        m, inv2 = host_prep(cfg, x[b], grid[b], grid_weight[b], edge_grid[b],
                            edge_Gauss[b], basepts, base_weight, D, weights)
        maps.append(m)
        invs.append(inv2)
    return maps, invs


def finish(cfg, out_tbl, p_newrow):
    # device row p_newrow[pair] holds pair's output
    o = out_tbl[p_newrow]
    return np.ascontiguousarray(
        o.reshape(cfg["N"], 32)[:, :cfg["COUT"]].T)


_BUILT = {}


def _get_nc():
    if "nc" not in _BUILT:
        nc = bacc.Bacc("TRN2", target_bir_lowering=False,
                       dynamic_dma_scratch_size=32768,
                       num_swdge_queues=4)
        build(nc, CFG)
        nc.compile()
        _BUILT["nc"] = nc
    return _BUILT["nc"]


def kernel(x, grid, grid_weight, edge_grid, edge_Gauss, basepts, base_weight,
           D, weights, _trace=False):
    cfg = CFG
    in_maps, invs = make_in_maps(
        cfg, np.asarray(x, np.float32), np.asarray(grid),
        np.asarray(grid_weight), np.asarray(edge_grid),
        np.asarray(edge_Gauss), np.asarray(basepts),
        np.asarray(base_weight), np.asarray(D), np.asarray(weights))
    nc = _get_nc()
    res = bass_utils.run_bass_kernel_spmd(
        nc, in_maps, core_ids=list(range(x.shape[0])), trace=_trace)
    out = np.stack([finish(cfg, res.results[b]["out"], invs[b])
                    for b in range(x.shape[0])])
    kernel.last_result = res
    return out
